# revision 53
# baseline (speedup 1.0000x reference)
"""Trainium2 Bass kernel for nn_BatchHighOrderActivation.

Reference semantics (per batch b, channel g):
    sort the ARITY=4 values x = X[b,g,:], build barycentric coefficients from
    the sorted gaps, gather params rows by reverse-cumsum bitmasks, contract.

Sort/gather-free reformulation (multilinear simplex / Lovasz form):
    out[b,g,:] = sum_{m=0..15} relu(w[b,g,m]) * params'[g,m,:]
    w[m]  = min_{i in m} x_i - max_{i not in m} x_i     for m in 1..14
    w[15] = min_i x_i,  w[0] = -min_i x_i               (x = relu(x)-relu(-x))
    params'[g,0,:] = -params[g,15,:] so the m0/m15 pair reproduces the
    un-relu'd min_i x_i * params[g,15,:] term; every column is then relu'd
    uniformly, which lets the relu ride the PSUM->SBUF evacuation for free.

Kernel structure per core (pure batch data-parallel sharding, 512 rows/core),
fp16 internal compute AND fp16 output wire format (host upconverts):
  - host: X de-interleaved to fp16 arity-planes, two 128-row b-tiles packed
          per plane; params expanded to an fp16 block-diagonal table
          (8 channels/group, K-order (m,gl)) with row m=0 := -row m=15
  - DVE : subset min/max tree at FD=1024 (b-tile pairs; DVE per-op overhead
          is ~210 cyc so bigger contiguous ops win), then per b-tile the 14
          half-width strided subtractions + qmin/negate into W
  - PE  : transpose W 128x128 chunks via fp16 identity matmul into the last
          PSUM bank of the group's 4-bank tile (fp16 bitcast view)
  - DVE/ACT: W^T evacuation with FUSED RELU (tensor_scalar_max / Relu)
  - PE  : block-diagonal fp16 matmul (K=(m,gl)=128, N=8ch*32=256) overwrites
          the same 4-bank PSUM tile in fp32
  - DVE/ACT: single [128,2048] PSUM->SBUF output evacuation per group,
          casting fp32->fp16 (half the per-op overhead of [128,1024] evacs)
  - X loads ride the sync HWDGE ring (SP issues DMAs earliest after the
    engine-table preamble: first tree op starts ~10us sooner than SWDGE),
    the params table rides the scalar HWDGE ring in 4 chunks, and output
    stores ride sync behind the (long-finished) X loads
"""

import numpy as np
from contextlib import ExitStack

import concourse.bass as bass
import concourse.mybir as mybir
import concourse.tile as tile
from concourse import bacc
from concourse.bass_utils import run_bass_kernel_spmd
from concourse.masks import make_identity

F32 = mybir.dt.float32
F16 = mybir.dt.float16
NCORES = 8
B, G, A, O = 4096, 512, 4, 32
BS = B // NCORES        # 512 batch rows per core
NBT = BS // 128         # 4 b-tiles per core
NPAIR = NBT // 2        # b-tile pairs (tree computed at FD=2*G)
NQ = G // 8             # 64 channel groups of 8

_PAIRS = [(0, 1), (0, 2), (0, 3), (1, 2), (1, 3), (2, 3)]
_TRIPLES = [(0, 1, 2), (0, 1, 3), (0, 2, 3), (1, 2, 3)]
_SUBS = [3, 5, 9, 6, 10, 12, 7, 11, 13, 14, 1, 2, 4, 8]

_cached_nc = None


def _build_program():
    nc = bacc.Bacc("TRN2", target_bir_lowering=False, debug=False, num_devices=NCORES)

    # X pre-deinterleaved on host into fp16 planes, b-tile pairs packed
    # per row: row (pr*128+p) holds [a, t, g] for batch rows pr*256+t*128+p
    x_d = nc.dram_tensor("x", [NPAIR * 128, A * 2 * G], F16, kind="ExternalInput").ap()
    pbd_d = nc.dram_tensor("pbd", [128, NQ * 256], F16, kind="ExternalInput").ap()
    out_d = nc.dram_tensor("out", [BS, G * O], F16, kind="ExternalOutput").ap()

    RELU = mybir.ActivationFunctionType.Relu

    with ExitStack() as ctx:
        tc = ctx.enter_context(tile.TileContext(nc))
        persist = ctx.enter_context(tc.tile_pool(name="persist", bufs=1))
        plpool = ctx.enter_context(tc.tile_pool(name="pl", bufs=2))
        treep = ctx.enter_context(tc.tile_pool(name="tree", bufs=2))
        wpool = ctx.enter_context(tc.tile_pool(name="w", bufs=2))
        lhsp = ctx.enter_context(tc.tile_pool(name="lt", bufs=3))
        stgp = ctx.enter_context(tc.tile_pool(name="stg", bufs=2))
        ptp = ctx.enter_context(tc.tile_pool(name="pt", bufs=2, space="PSUM"))
        pmp = ctx.enter_context(tc.tile_pool(name="pm", bufs=3, space="PSUM"))

        # b-tile pairs: the 1024-wide tree ops amortize DVE's ~210cyc per-op
        # overhead; measured best among [0][1,2][3], [0][1][2,3], solo
        groups = [[0, 1], [2, 3]]

        pbd = [
            persist.tile([128, 16 * 256], F16, name=f"pbd{i}") for i in range(4)
        ]
        identity = persist.tile([128, 128], F16)
        # first X group on the sync HWDGE ring: SP clears its engine-table
        # preamble first, so this is the earliest possible load dispatch
        # pl0 loads one DMA per arity plane: the first tree ops need only
        # planes 0/1, so DVE starts ~4us sooner than with one 1MB load
        pl0 = plpool.tile([128, A, 2, G], F16, tag="pl")
        for i in range(A):
            nc.sync.dma_start(
                pl0[:, i, :, :].rearrange("p t g -> p (t g)"),
                x_d[0:128, i * 2 * G:(i + 1) * 2 * G],
            )
        for i in range(4):
            nc.scalar.dma_start(pbd[i][:], pbd_d[:, i * 4096:(i + 1) * 4096])
        make_identity(nc, identity[:])

        ev = [0]
        oev = [0]
        for gi, grp in enumerate(groups):
            gw = len(grp)
            if gi == 0:
                pl = pl0
            else:
                pl = plpool.tile([128, A, 2, G], F16, tag="pl", name=f"pl{gi}")
                nc.sync.dma_start(
                    pl[:].rearrange("p a t g -> p (a t g)"),
                    x_d[gi * 128:(gi + 1) * 128, :],
                )
            # group-wide arity planes [128, gw*G]
            s2 = [pl[:, i, :, :] for i in range(A)]

            tr = treep.tile([128, 20, gw, G], F16, tag="tree", name=f"tr{gi}")
            slot = [0]
            mn, mx = {}, {}

            def alloc():
                ap = tr[:, slot[0], :, :]
                slot[0] += 1
                return ap

            for (i, j) in _PAIRS:
                mn[(i, j)] = alloc()
                nc.vector.tensor_tensor(mn[(i, j)], s2[i], s2[j], mybir.AluOpType.min)
            for (i, j) in _PAIRS:
                mx[(i, j)] = alloc()
                nc.vector.tensor_tensor(mx[(i, j)], s2[i], s2[j], mybir.AluOpType.max)
            for (i, j, k) in _TRIPLES:
                mn[(i, j, k)] = alloc()
                nc.vector.tensor_tensor(mn[(i, j, k)], mn[(i, j)], s2[k], mybir.AluOpType.min)
                mx[(i, j, k)] = alloc()
                nc.vector.tensor_tensor(mx[(i, j, k)], mx[(i, j)], s2[k], mybir.AluOpType.max)

            def sub_ap(S):
                return s2[S[0]] if len(S) == 1 else mn[S]

            def sup_ap(Cm):
                return s2[Cm[0]] if len(Cm) == 1 else mx[Cm]

            for bt2, bt in enumerate(grp):
                last_bt = bt == NBT - 1
                # W layout: free = q*128 + m*8 + gl (K-order (m,gl)); sub
                # writes are 32-run strided ops - the measured sweet spot
                # (286ns vs 810ns for 64-run full-width).
                wt = wpool.tile([128, NQ * 128], F16, tag="w")
                wv4 = wt.rearrange("p (q m gl) -> p q m gl", m=16, gl=8)

                for hf2 in range(2):
                    qh = slice(hf2 * 32, hf2 * 32 + 32)
                    gh = slice(hf2 * 256, hf2 * 256 + 256)
                    # qmin lands directly in W's m15 column
                    nc.vector.tensor_tensor(
                        wv4[:, qh, 15, :],
                        mn[(0, 1, 2)][:, bt2, gh], s2[3][:, bt2, gh],
                        mybir.AluOpType.min,
                    )
                    # m0 column: -qmin (x = relu(x)-relu(-x) paired with the
                    # negated m=15 params rows): uniformly relu-able W
                    nc.vector.tensor_scalar_mul(
                        wv4[:, qh, 0, :], wv4[:, qh, 15, :], -1.0
                    )
                    for m in _SUBS:
                        S = tuple(i for i in range(A) if (m >> i) & 1)
                        Cm = tuple(i for i in range(A) if not ((m >> i) & 1))
                        nc.vector.tensor_tensor(
                            wv4[:, qh, m, :],
                            sub_ap(S)[:, bt2, gh],
                            sup_ap(Cm)[:, bt2, gh],
                            mybir.AluOpType.subtract,
                        )

                for hf in range(2):
                    stg = stgp.tile([128, 32 * 256], F16, tag="stg")
                    for gp in range(2):
                        for gqi in range(2):
                            q0 = hf * 32 + gp * 16 + gqi * 8
                            pt = ptp.tile([128, 8 * 128], F16, tag="pt")
                            for j in range(8):
                                q = q0 + j
                                nc.tensor.transpose(
                                    pt[:, j * 128:(j + 1) * 128],
                                    wt[:, q * 128:(q + 1) * 128],
                                    identity[:],
                                )
                            lt = lhsp.tile([128, 8 * 128], F16, tag="lt")
                            # W^T evacuation with fused relu on DVE (fp16 2x
                            # mode, ~1.7x cheaper there than on ACT)
                            nc.vector.tensor_scalar_max(lt[:], pt[:], 0.0)
                            ev[0] += 1
                            for half in range(2):
                                pm = pmp.tile([128, 1024], F32, tag="pm")
                                for j2 in range(4):
                                    j = half * 4 + j2
                                    qq = q0 + j
                                    nc.tensor.matmul(
                                        pm[:, j2 * 256:(j2 + 1) * 256],
                                        lt[:, j * 128:(j + 1) * 128],
                                        pbd[qq // 16][:, (qq % 16) * 256:(qq % 16 + 1) * 256],
                                        start=True,
                                        stop=True,
                                    )
                                dst = stg[:, gp * 4096 + gqi * 2048 + half * 1024:
                                          gp * 4096 + gqi * 2048 + (half + 1) * 1024]
                                # out evacuation fp32->fp16: ACT, except ~1/3
                                # on DVE on the tree-free last b-tile
                                dve_out = (oev[0] % 3 == 1) if last_bt else False
                                if dve_out:
                                    nc.vector.tensor_copy(dst, pm[:])
                                else:
                                    nc.scalar.copy(dst, pm[:])
                                oev[0] += 1
                            if last_bt:
                                # finer 0.5MB stores at the drain tail
                                nc.sync.dma_start(
                                    out_d[bt * 128:(bt + 1) * 128,
                                          q0 * 256:(q0 + 8) * 256],
                                    stg[:, gp * 4096 + gqi * 2048:
                                        gp * 4096 + (gqi + 1) * 2048],
                                )
                    if not last_bt:
                        nc.sync.dma_start(
                            out_d[bt * 128:(bt + 1) * 128,
                                  hf * 8192:(hf + 1) * 8192],
                            stg[:],
                        )

    nc.compile()
    return nc


def _get_program():
    global _cached_nc
    if _cached_nc is None:
        _cached_nc = _build_program()
    return _cached_nc


def _make_inputs(X, params):
    X = np.ascontiguousarray(X, dtype=np.float32)
    params = np.ascontiguousarray(params, dtype=np.float32)
    P4 = params.reshape(NQ, 8, 16, O)                 # [q, gl, m, o]
    # block-diag table: pbd[m*8+gl, q*256 + gl*32 + o] = params[8q+gl, m, o]
    # row m=0 carries -params[...,15,:] (pairs with the -qmin W column)
    Pb = np.zeros((16, 8, NQ, 8, O), np.float32)
    for gl in range(8):
        Pb[1:, gl, :, gl, :] = P4[:, gl, 1:, :].transpose(1, 0, 2)
        Pb[0, gl, :, gl, :] = -P4[:, gl, 15, :]
    pbd = np.ascontiguousarray(Pb.reshape(128, NQ * 256).astype(np.float16))
    # de-interleave X to per-arity fp16 planes, packing b-tile pairs:
    # xp[c, pr, p, a, t, g] = X[c*BS + pr*256 + t*128 + p, g, a]
    Xp = (X.reshape(NCORES, NBT // 2, 2, 128, G, A)
            .transpose(0, 1, 3, 5, 2, 4)              # c, pr, p, a, t, g
            .astype(np.float16))
    Xp = np.ascontiguousarray(Xp.reshape(NCORES, NPAIR * 128, A * 2 * G))
    in_maps = [
        {"x": Xp[c], "pbd": pbd}
        for c in range(NCORES)
    ]
    return in_maps


def kernel(X, params):
    nc = _get_program()
    in_maps = _make_inputs(X, params)
    res = run_bass_kernel_spmd(nc, in_maps, list(range(NCORES))).results
    out = np.concatenate(
        [res[c]["out"].astype(np.float32).reshape(BS, G, O) for c in range(NCORES)],
        axis=0,
    )
    return out


def kernel_traced(X, params):
    """Like kernel() but also returns the BassKernelResults (profile info)."""
    nc = _get_program()
    in_maps = _make_inputs(X, params)
    br = run_bass_kernel_spmd(nc, in_maps, list(range(NCORES)), trace=True)
    out = np.concatenate(
        [br.results[c]["out"].astype(np.float32).reshape(BS, G, O)
         for c in range(NCORES)],
        axis=0,
    )
    return out, br


# revision 57
# speedup vs baseline: 1.0598x; 1.0598x over previous
"""Trainium2 Bass kernel for nn_BatchHighOrderActivation.

Reference semantics (per batch b, channel g):
    sort the ARITY=4 values x = X[b,g,:], build barycentric coefficients from
    the sorted gaps, gather params rows by reverse-cumsum bitmasks, contract.

Sort/gather-free reformulation (multilinear simplex / Lovasz form):
    out[b,g,:] = sum_{m=0..15} relu(w[b,g,m]) * params'[g,m,:]
    w[m]  = min_{i in m} x_i - max_{i not in m} x_i     for m in 1..14
    w[15] = min_i x_i,  w[0] = -min_i x_i               (x = relu(x)-relu(-x))
    params'[g,0,:] = -params[g,15,:] so the m0/m15 pair reproduces the
    un-relu'd min_i x_i * params[g,15,:] term; every column is then relu'd
    uniformly, which lets the relu ride the PSUM->SBUF evacuation for free.

Kernel structure per core (pure batch data-parallel sharding, 512 rows/core),
fp16 internal compute AND fp16 output wire format (host upconverts):
  - host: X de-interleaved to fp16 arity-planes, two 128-row b-tiles packed
          per plane; params expanded to an fp16 block-diagonal table
          (8 channels/group, K-order (m,gl)) with row m=0 := -row m=15
  - DVE : subset min/max tree at FD=1024 (b-tile pairs; DVE per-op overhead
          is ~210 cyc so bigger contiguous ops win), then per b-tile the 14
          half-width strided subtractions + qmin/negate into W
  - PE  : transpose W 128x128 chunks via fp16 identity matmul into the last
          PSUM bank of the group's 4-bank tile (fp16 bitcast view)
  - DVE/ACT: W^T evacuation with FUSED RELU (tensor_scalar_max / Relu)
  - PE  : block-diagonal fp16 matmul (K=(m,gl)=128, N=8ch*32=256) overwrites
          the same 4-bank PSUM tile in fp32
  - DVE/ACT: single [128,2048] PSUM->SBUF output evacuation per group,
          casting fp32->fp16 (half the per-op overhead of [128,1024] evacs)
  - X loads ride the sync HWDGE ring (SP issues DMAs earliest after the
    engine-table preamble: first tree op starts ~4us sooner than SWDGE),
    the params table rides the scalar HWDGE ring in 4 chunks, and output
    stores ride sync behind the (long-finished) X loads

Measured on 8-core SPMD axon trn2: 106.5-107.2us (vs 133.2us staged
baseline; engines: DVE ~75us busy, ACT ~67, PE ~54, DMA ~70 of a ~112us
window). Remaining gaps: ~12us pipeline-fill head (input-load latency +
bt0's serial tree/sub phase) and ~13us drain tail (bt3's evacuation and
final stores), both partially irreducible at this dependency depth.
"""

import numpy as np
from contextlib import ExitStack

import concourse.bass as bass
import concourse.mybir as mybir
import concourse.tile as tile
from concourse import bacc
from concourse.bass_utils import run_bass_kernel_spmd
from concourse.masks import make_identity

F32 = mybir.dt.float32
F16 = mybir.dt.float16
NCORES = 8
B, G, A, O = 4096, 512, 4, 32
BS = B // NCORES        # 512 batch rows per core
NBT = BS // 128         # 4 b-tiles per core
NPAIR = NBT // 2        # b-tile pairs (tree computed at FD=2*G)
NQ = G // 8             # 64 channel groups of 8

_PAIRS = [(0, 1), (0, 2), (0, 3), (1, 2), (1, 3), (2, 3)]
_TRIPLES = [(0, 1, 2), (0, 1, 3), (0, 2, 3), (1, 2, 3)]
_SUBS = [3, 5, 9, 6, 10, 12, 7, 11, 13, 14, 1, 2, 4, 8]

_cached_nc = None


def _build_program():
    nc = bacc.Bacc("TRN2", target_bir_lowering=False, debug=False, num_devices=NCORES)

    # X pre-deinterleaved on host into fp16 planes, b-tile pairs packed
    # per row: row (pr*128+p) holds [a, t, g] for batch rows pr*256+t*128+p
    x_d = nc.dram_tensor("x", [NPAIR * 128, A * 2 * G], F16, kind="ExternalInput").ap()
    pbd_d = nc.dram_tensor("pbd", [128, NQ * 256], F16, kind="ExternalInput").ap()
    out_d = nc.dram_tensor("out", [BS, G * O], F16, kind="ExternalOutput").ap()

    RELU = mybir.ActivationFunctionType.Relu

    with ExitStack() as ctx:
        tc = ctx.enter_context(tile.TileContext(nc))
        persist = ctx.enter_context(tc.tile_pool(name="persist", bufs=1))
        plpool = ctx.enter_context(tc.tile_pool(name="pl", bufs=2))
        treep = ctx.enter_context(tc.tile_pool(name="tree", bufs=2))
        wpool = ctx.enter_context(tc.tile_pool(name="w", bufs=2))
        lhsp = ctx.enter_context(tc.tile_pool(name="lt", bufs=3))
        stgp = ctx.enter_context(tc.tile_pool(name="stg", bufs=2))
        ptp = ctx.enter_context(tc.tile_pool(name="pt", bufs=2, space="PSUM"))
        pmp = ctx.enter_context(tc.tile_pool(name="pm", bufs=3, space="PSUM"))

        # b-tile pairs: the 1024-wide tree ops amortize DVE's ~210cyc per-op
        # overhead; measured best among [0][1,2][3], [0][1][2,3], solo
        groups = [[0, 1], [2, 3]]

        pbd = [
            persist.tile([128, 16 * 256], F16, name=f"pbd{i}") for i in range(4)
        ]
        identity = persist.tile([128, 128], F16)
        # first X group on the sync HWDGE ring: SP clears its engine-table
        # preamble first, so this is the earliest possible load dispatch
        pl0 = plpool.tile([128, A, 2, G], F16, tag="pl")
        nc.sync.dma_start(pl0[:].rearrange("p a t g -> p (a t g)"), x_d[0:128, :])
        for i in range(4):
            nc.scalar.dma_start(pbd[i][:], pbd_d[:, i * 4096:(i + 1) * 4096])
        make_identity(nc, identity[:])

        ev = [0]
        oev = [0]
        for gi, grp in enumerate(groups):
            gw = len(grp)
            if gi == 0:
                pl = pl0
            else:
                pl = plpool.tile([128, A, 2, G], F16, tag="pl", name=f"pl{gi}")
                nc.sync.dma_start(
                    pl[:].rearrange("p a t g -> p (a t g)"),
                    x_d[gi * 128:(gi + 1) * 128, :],
                )
            # group-wide arity planes [128, gw*G]
            s2 = [pl[:, i, :, :] for i in range(A)]

            tr = treep.tile([128, 20, gw, G], F16, tag="tree", name=f"tr{gi}")
            slot = [0]
            mn, mx = {}, {}

            def alloc():
                ap = tr[:, slot[0], :, :]
                slot[0] += 1
                return ap

            for (i, j) in _PAIRS:
                mn[(i, j)] = alloc()
                nc.vector.tensor_tensor(mn[(i, j)], s2[i], s2[j], mybir.AluOpType.min)
            for (i, j) in _PAIRS:
                mx[(i, j)] = alloc()
                nc.vector.tensor_tensor(mx[(i, j)], s2[i], s2[j], mybir.AluOpType.max)
            for (i, j, k) in _TRIPLES:
                mn[(i, j, k)] = alloc()
                nc.vector.tensor_tensor(mn[(i, j, k)], mn[(i, j)], s2[k], mybir.AluOpType.min)
                mx[(i, j, k)] = alloc()
                nc.vector.tensor_tensor(mx[(i, j, k)], mx[(i, j)], s2[k], mybir.AluOpType.max)

            def sub_ap(S):
                return s2[S[0]] if len(S) == 1 else mn[S]

            def sup_ap(Cm):
                return s2[Cm[0]] if len(Cm) == 1 else mx[Cm]

            for bt2, bt in enumerate(grp):
                last_bt = bt == NBT - 1
                # W layout: free = q*128 + m*8 + gl (K-order (m,gl)); sub
                # writes are 32-run strided ops - the measured sweet spot
                # (286ns vs 810ns for 64-run full-width).
                wt = wpool.tile([128, NQ * 128], F16, tag="w")
                wv4 = wt.rearrange("p (q m gl) -> p q m gl", m=16, gl=8)

                for hf2 in range(2):
                    qh = slice(hf2 * 32, hf2 * 32 + 32)
                    gh = slice(hf2 * 256, hf2 * 256 + 256)
                    # qmin lands directly in W's m15 column
                    nc.vector.tensor_tensor(
                        wv4[:, qh, 15, :],
                        mn[(0, 1, 2)][:, bt2, gh], s2[3][:, bt2, gh],
                        mybir.AluOpType.min,
                    )
                    # m0 column: -qmin (x = relu(x)-relu(-x) paired with the
                    # negated m=15 params rows): uniformly relu-able W
                    nc.vector.tensor_scalar_mul(
                        wv4[:, qh, 0, :], wv4[:, qh, 15, :], -1.0
                    )
                    for m in _SUBS:
                        S = tuple(i for i in range(A) if (m >> i) & 1)
                        Cm = tuple(i for i in range(A) if not ((m >> i) & 1))
                        nc.vector.tensor_tensor(
                            wv4[:, qh, m, :],
                            sub_ap(S)[:, bt2, gh],
                            sup_ap(Cm)[:, bt2, gh],
                            mybir.AluOpType.subtract,
                        )

                for hf in range(2):
                    stg = stgp.tile([128, 32 * 256], F16, tag="stg")
                    for gp in range(2):
                        for gqi in range(2):
                            q0 = hf * 32 + gp * 16 + gqi * 8
                            pt = ptp.tile([128, 8 * 128], F16, tag="pt")
                            for j in range(8):
                                q = q0 + j
                                nc.tensor.transpose(
                                    pt[:, j * 128:(j + 1) * 128],
                                    wt[:, q * 128:(q + 1) * 128],
                                    identity[:],
                                )
                            lt = lhsp.tile([128, 8 * 128], F16, tag="lt")
                            # W^T evacuation with fused relu on DVE (fp16 2x
                            # mode, ~1.7x cheaper there than on ACT)
                            nc.vector.tensor_scalar_max(lt[:], pt[:], 0.0)
                            ev[0] += 1
                            for half in range(2):
                                pm = pmp.tile([128, 1024], F32, tag="pm")
                                for j2 in range(4):
                                    j = half * 4 + j2
                                    qq = q0 + j
                                    nc.tensor.matmul(
                                        pm[:, j2 * 256:(j2 + 1) * 256],
                                        lt[:, j * 128:(j + 1) * 128],
                                        pbd[qq // 16][:, (qq % 16) * 256:(qq % 16 + 1) * 256],
                                        start=True,
                                        stop=True,
                                    )
                                dst = stg[:, gp * 4096 + gqi * 2048 + half * 1024:
                                          gp * 4096 + gqi * 2048 + (half + 1) * 1024]
                                # out evacuation fp32->fp16: ACT, except a
                                # few on DVE on the tree-free last b-tile
                                dve_out = (oev[0] % 5 == 2) if last_bt else False
                                if dve_out:
                                    nc.vector.tensor_copy(dst, pm[:])
                                else:
                                    nc.scalar.copy(dst, pm[:])
                                oev[0] += 1
                        if last_bt:
                            # finer stores at the drain tail
                            qq0 = hf * 32 + gp * 16
                            nc.sync.dma_start(
                                out_d[bt * 128:(bt + 1) * 128,
                                      qq0 * 256:(qq0 + 16) * 256],
                                stg[:, gp * 4096:(gp + 1) * 4096],
                            )
                    if not last_bt:
                        nc.sync.dma_start(
                            out_d[bt * 128:(bt + 1) * 128,
                                  hf * 8192:(hf + 1) * 8192],
                            stg[:],
                        )

    nc.compile()
    return nc


def _get_program():
    global _cached_nc
    if _cached_nc is None:
        _cached_nc = _build_program()
    return _cached_nc


def _make_inputs(X, params):
    X = np.ascontiguousarray(X, dtype=np.float32)
    params = np.ascontiguousarray(params, dtype=np.float32)
    P4 = params.reshape(NQ, 8, 16, O)                 # [q, gl, m, o]
    # block-diag table: pbd[m*8+gl, q*256 + gl*32 + o] = params[8q+gl, m, o]
    # row m=0 carries -params[...,15,:] (pairs with the -qmin W column)
    Pb = np.zeros((16, 8, NQ, 8, O), np.float32)
    for gl in range(8):
        Pb[1:, gl, :, gl, :] = P4[:, gl, 1:, :].transpose(1, 0, 2)
        Pb[0, gl, :, gl, :] = -P4[:, gl, 15, :]
    pbd = np.ascontiguousarray(Pb.reshape(128, NQ * 256).astype(np.float16))
    # de-interleave X to per-arity fp16 planes, packing b-tile pairs:
    # xp[c, pr, p, a, t, g] = X[c*BS + pr*256 + t*128 + p, g, a]
    Xp = (X.reshape(NCORES, NBT // 2, 2, 128, G, A)
            .transpose(0, 1, 3, 5, 2, 4)              # c, pr, p, a, t, g
            .astype(np.float16))
    Xp = np.ascontiguousarray(Xp.reshape(NCORES, NPAIR * 128, A * 2 * G))
    in_maps = [
        {"x": Xp[c], "pbd": pbd}
        for c in range(NCORES)
    ]
    return in_maps


def kernel(X, params):
    nc = _get_program()
    in_maps = _make_inputs(X, params)
    res = run_bass_kernel_spmd(nc, in_maps, list(range(NCORES))).results
    out = np.concatenate(
        [res[c]["out"].astype(np.float32).reshape(BS, G, O) for c in range(NCORES)],
        axis=0,
    )
    return out


def kernel_traced(X, params):
    """Like kernel() but also returns the BassKernelResults (profile info)."""
    nc = _get_program()
    in_maps = _make_inputs(X, params)
    br = run_bass_kernel_spmd(nc, in_maps, list(range(NCORES)), trace=True)
    out = np.concatenate(
        [br.results[c]["out"].astype(np.float32).reshape(BS, G, O)
         for c in range(NCORES)],
        axis=0,
    )
    return out, br
